# revision 48
# baseline (speedup 1.0000x reference)
"""FP16-pulse -> FP8(E4M3)-pulse converter as a Trainium2 Bass/Tile kernel. v9.4.

Input : fp16_pulse [4096, 4096, 16] f32 of 0/1 bits, [S, E4..E0, M9..M0] MSB first.
Output: [4096, 4096, 8] f32 of 0/1 bits, [S, E3..E0, M2..M0].

The pulse planes are packed losslessly into one uint16 per value on the host
(pure layout transform), the device performs the entire numeric conversion on
the packed values, and the host unpacks the result byte back into planes.
This cuts HBM traffic from 96 B/value to 6 B/value; the v8 kernel sat on the
f32 DMA roofline (559us busy/core), v9 is compute-bound at ~1/9th the time.

Device math (exact for every u = s<<15 | e<<10 | m; validated exhaustively
over all 2^16 patterns on device):
  em  = u & 0x7C00                    # 1024*e
  w2  = (u & 0x3FF) | 0x2400          # f16 bits of (1024+m)*2^-7 pre-scale
  sb2 = clamp(em, 4096, 9216)         # 1024*clamp(e,4,9)
  yv  = bitcast_f16(w2 + sb2)         # (1024+m) * 2^(clamp(e,4,9)-16), exact
  q   = RNE(yv)                       # fp32-internal MAGIC add/sub
  obr = relu(em/128 - 72)             # = relu(8e-72)
  ovf = 126*(e>22)                    # any value >= 6 works; min saturates
  B   = min(q + obr + ovf, 126) + 128*s
Key identities: 8*oe+om == 8*(e-8) + RNE(m/128) for normals (the mantissa
carry is absorbed by the byte encoding); clamp low bound 4 (not 5) makes
every e<5 input round to exactly 0 (no underflow mask); q >= 8 when e > 22 so
adding ovf pushes the byte past 126 and the min saturates it to exactly 126.

Schedule (TimelineSim 53.1us/core; DVE and ACT both ~43us busy and dense):
DVE keeps the bitwise field extractions (DVE-only ops), the two 2x
tensor_tensor adds, and the saturating min; the otherwise-idle PE sums
q + obr + ovf into PSUM via identity matmuls (fp32 accumulate, exact for
these small ints); ACT runs the relu masks and the PSUM->SBUF copy; Pool
runs the float MAGIC round plus small column shares of ovf (1/8) and of
the sign term (1/8, computed as (u<0)*128 so it is a float op Pool can
run). The shares balance DVE and ACT exactly; Pool is 5x slower per
element than DVE's 4x mode and saturates right at this load, so larger
shares regress. Emission is software-pipelined 4 deep
(front/matmul/copy/back) because each engine executes its queue in order
(3-deep couples PE to ACT and serializes); tiles are [896]+[1792]*8+[1152]
- the beat of ten slightly-small tiles schedules tighter than 8x2048; all
input DMAs are issued up-front (tile 0 first, then the PE identity) so
output-DMA sem-waits (which hold the SP sequencer) cannot delay loads.
"""

import numpy as np
from contextlib import ExitStack

import concourse.bass as bass
import concourse.bacc as bacc
import concourse.tile as tile
from concourse import mybir
from concourse.bass_utils import run_bass_kernel_spmd

F32 = mybir.dt.float32
F16 = mybir.dt.float16
I16 = mybir.dt.int16
OP = mybir.AluOpType
ACTF = mybir.ActivationFunctionType

P = 128
N_CORES = 8
B0, B1 = 4096, 4096
NBITS, OBITS = 16, 8
MAGIC = 12582912.0  # 1.5 * 2**23: fp32 add+sub rounds to nearest int (RNE)

VALS_PER_PART = (B0 // N_CORES) * B1 // P  # 16384

IDENT = np.eye(P, dtype=np.float16)


def _sizes(total: int) -> list[int]:
    if total < 8192:
        n = max(1, total // 512)
        return [total // n] * n
    body = (total - 2048) // 1792
    rem = total - 2048 - body * 1792
    return [896] + [1792] * body + ([rem] if rem else []) + [1152]


def build_nc(total: int, sizes=None, ovf_split=0.25, bm_split=1.0,
             dve_tail=0, q_eng="P", ovf_pool=0.125, tail_fast=False,
             s128_pool=0.125) -> bass.Bass:
    nc = bacc.Bacc()
    x = nc.declare_dram_parameter("x", [P, total], I16, isOutput=False)
    ident_d = nc.declare_dram_parameter("ident", [P, P], F16, isOutput=False)
    y = nc.declare_dram_parameter("y", [P, total], I16, isOutput=True)

    sizes = sizes or _sizes(total)
    offs = np.concatenate([[0], np.cumsum(sizes)]).tolist()
    ntiles = len(sizes)

    with tile.TileContext(nc) as tc, ExitStack() as ctx:
        iop = ctx.enter_context(tc.tile_pool(name="io", bufs=2))
        tp = ctx.enter_context(tc.tile_pool(name="tmp", bufs=2))
        pp = ctx.enter_context(tc.tile_pool(name="ps", bufs=2, space="PSUM"))

        V, G, S = nc.vector, nc.gpsimd, nc.scalar

        b72 = tp.tile([P, 1], F32, tag="b72", name="b72", bufs=1)
        G.memset(b72[:], -72.0)
        b132 = tp.tile([P, 1], F32, tag="b132", name="b132", bufs=1)
        G.memset(b132[:], -132.0)

        # prefetch every input tile before any compute is issued; tile 0
        # first (it gates the pipeline), then ident (PE needs it one stage
        # later), then the rest
        us = []
        for t in range(ntiles):
            u = iop.tile([P, sizes[t]], I16, tag=f"u{t}", name="u", bufs=1)
            us.append(u)
        nc.sync.dma_start(us[0][:], x[:, offs[0]:offs[1]])
        ident = tp.tile([P, P], F16, tag="ident", name="ident", bufs=1)
        nc.sync.dma_start(ident[:], ident_d[:])
        for t in range(1, ntiles):
            nc.sync.dma_start(us[t][:], x[:, offs[t]:offs[t + 1]])

        state: dict = {}

        def vt(tag, w, dt=I16, bufs=3):
            return tp.tile([P, w], dt, tag=tag, name=tag, bufs=bufs)

        def tile_inline(t):
            """Whole chain for one (small) tile with minimal cross-engine
            hops: used for the last tile so the pipeline drain is a short
            DVE-local dependency chain instead of a 4-engine round trip."""
            w = sizes[t]
            u = us[t]
            em = vt("em", w, bufs=2)
            V.tensor_scalar(em[:], u[:], 0x7C00, None, OP.bitwise_and)
            obr = vt("obr", w, F16, bufs=4)
            S.activation(obr[:], em[:], ACTF.Relu, bias=b72[:],
                         scale=0.0078125)
            ovf = vt("ovf", w, F16, bufs=4)
            V.tensor_scalar(ovf[:], em[:], 22528, 126, OP.is_gt, OP.mult)
            w2 = vt("w2", w, bufs=2)
            V.tensor_scalar(w2[:], u[:], 0x3FF, 0x2400,
                            OP.bitwise_and, OP.bitwise_or)
            sb2 = vt("sb2", w, bufs=2)
            V.tensor_scalar(sb2[:], em[:], 9216, 4096, OP.min, OP.max)
            yvb = vt("yvb", w, bufs=3)
            V.tensor_tensor(yvb[:], w2[:], sb2[:], OP.add)
            q = vt("q", w, F16, bufs=4)
            V.tensor_scalar(q[:], yvb[:].bitcast(F16), MAGIC, MAGIC,
                            OP.add, OP.subtract)
            L = vt("L", w, F16, bufs=2)
            V.tensor_tensor(L[:], q[:], obr[:], OP.add)
            lp = vt("Lp", w, F16, bufs=2)
            V.tensor_tensor(lp[:], L[:], ovf[:], OP.add)
            bm = vt("bm", w, bufs=3)
            V.tensor_scalar(bm[:], lp[:], 126, 0, OP.min, OP.add)
            s128 = vt("s128", w, bufs=5)
            V.tensor_scalar(s128[:], u[:], 8, 128,
                            OP.logical_shift_right, OP.bitwise_and)
            B = iop.tile([P, w], I16, tag="B", name="B", bufs=3)
            V.tensor_tensor(B[:], bm[:], s128[:], OP.add)
            nc.sync.dma_start(y[:, offs[t]:offs[t + 1]], B[:])
            state[t] = {"done": True}

        def stage_front(t):
            if tail_fast and t == ntiles - 1:
                tile_inline(t)
                return
            w = sizes[t]
            u = us[t]
            # even 128-col split points for the shared ops
            c_ovf = int(w * ovf_split + 127) // 128 * 128  # DVE part / ACT part
            em = vt("em", w, bufs=2)
            V.tensor_scalar(em[:], u[:], 0x7C00, None, OP.bitwise_and)
            w2 = vt("w2", w, bufs=2)
            V.tensor_scalar(w2[:], u[:], 0x3FF, 0x2400,
                            OP.bitwise_and, OP.bitwise_or)
            if s128_pool > 0:
                # sign as a float op, (u<0)*128, so Pool can take a share
                s128 = vt("s128", w, F16, bufs=5)
                c_s = int(w * s128_pool + 127) // 128 * 128
                G.tensor_scalar(s128[:, :c_s], u[:, :c_s], 0, 128,
                                OP.is_lt, OP.mult)
                if c_s < w:
                    V.tensor_scalar(s128[:, c_s:], u[:, c_s:], 0, 128,
                                    OP.is_lt, OP.mult)
            else:
                s128 = vt("s128", w, bufs=5)
                V.tensor_scalar(s128[:], u[:], 8, 128,
                                OP.logical_shift_right, OP.bitwise_and)
            obr = vt("obr", w, F16, bufs=4)
            S.activation(obr[:], em[:], ACTF.Relu, bias=b72[:],
                         scale=0.0078125)
            ovf = vt("ovf", w, F16, bufs=4)
            c_ovp = c_ovf + int(w * ovf_pool + 127) // 128 * 128
            c_ovp = min(c_ovp, w)
            if c_ovf > 0:
                V.tensor_scalar(ovf[:, :c_ovf], em[:, :c_ovf], 22528, 126,
                                OP.is_gt, OP.mult)
            if c_ovp > c_ovf:
                G.tensor_scalar(ovf[:, c_ovf:c_ovp], em[:, c_ovf:c_ovp],
                                22528, 126, OP.is_gt, OP.mult)
            if c_ovp < w:
                S.activation(ovf[:, c_ovp:], em[:, c_ovp:], ACTF.Relu,
                             bias=b132[:], scale=0.005859375)
            sb2 = vt("sb2", w, bufs=2)
            V.tensor_scalar(sb2[:], em[:], 9216, 4096, OP.min, OP.max)
            yvb = vt("yvb", w, bufs=3)
            V.tensor_tensor(yvb[:], w2[:], sb2[:], OP.add)
            q = vt("q", w, F16, bufs=4)
            (G if q_eng == "P" else V).tensor_scalar(
                q[:], yvb[:].bitcast(F16), MAGIC, MAGIC,
                OP.add, OP.subtract)
            state[t] = {"s128": s128, "obr": obr, "ovf": ovf, "q": q}

        def stage_matmul(t):
            st = state[t]
            if st.get("done"):
                return
            w = sizes[t]
            if t >= ntiles - dve_tail:
                # fast drain: the last tile(s) bypass PE+copy so the tail of
                # the pipeline is a short DVE-only chain
                L = vt("L", w, F16, bufs=2)
                V.tensor_tensor(L[:], st["q"][:], st["obr"][:], OP.add)
                lp = vt("Lp", w, F16, bufs=2)
                V.tensor_tensor(lp[:], L[:], st["ovf"][:], OP.add)
                st["lpf"] = lp
                return
            ps = pp.tile([P, w], F32, tag="ps", name="ps", bufs=2)
            for c0 in range(0, w, 512):
                c1 = min(c0 + 512, w)
                sl = (slice(None), slice(c0, c1))
                nc.tensor.matmul(ps[sl], ident[:], st["q"][sl],
                                 start=True, stop=False)
                nc.tensor.matmul(ps[sl], ident[:], st["obr"][sl],
                                 start=False, stop=False)
                nc.tensor.matmul(ps[sl], ident[:], st["ovf"][sl],
                                 start=False, stop=True)
            st["ps"] = ps

        def stage_copy(t):
            st = state[t]
            if st.get("done"):
                return
            w = sizes[t]
            c_bm = int(w * bm_split + 127) // 128 * 128  # DVE part / Pool part
            if "lpf" in st:
                lpf = st["lpf"]
            else:
                lpf = vt("lpf", w, F16, bufs=3)
                S.activation(lpf[:], st["ps"][:], ACTF.Copy)
            bm = vt("bm", w, bufs=3)
            if c_bm > 0:
                V.tensor_scalar(bm[:, :c_bm], lpf[:, :c_bm], 126, 0,
                                OP.min, OP.add)
            if c_bm < w:
                G.tensor_scalar(bm[:, c_bm:], lpf[:, c_bm:], 126, 0,
                                OP.min, OP.add)
            st["bm"] = bm

        def stage_back(t):
            st = state.pop(t)
            if st.get("done"):
                return
            w = sizes[t]
            B = iop.tile([P, w], I16, tag="B", name="B", bufs=3)
            V.tensor_tensor(B[:], st["bm"][:], st["s128"][:], OP.add)
            nc.sync.dma_start(y[:, offs[t]:offs[t + 1]], B[:])

        # software-pipelined emission: per-engine queues are in-order, so
        # interleave tile t's front with t-1's matmul, t-2's copy and t-3's
        # back to keep every engine's next instruction dependency-satisfied.
        for t in range(ntiles + 3):
            if t < ntiles:
                stage_front(t)
            if 1 <= t < ntiles + 1:
                stage_matmul(t - 1)
            if 2 <= t < ntiles + 2:
                stage_copy(t - 2)
            if t >= 3:
                stage_back(t - 3)
    nc.compile()
    return nc


_NC_CACHE: dict = {}


def _get_nc(total: int) -> bass.Bass:
    if total not in _NC_CACHE:
        _NC_CACHE[total] = build_nc(total)
    return _NC_CACHE[total]


def kernel(fp16_pulse: np.ndarray) -> np.ndarray:
    assert fp16_pulse.shape == (B0, B1, NBITS)
    in_dtype = fp16_pulse.dtype
    # lossless layout packing: 16 pulse planes -> one uint16 per value
    bits = np.ascontiguousarray(fp16_pulse).astype(np.uint8)
    packed = np.packbits(bits.reshape(-1, NBITS), axis=-1)  # [N, 2] big-endian
    u16 = packed.view(">u2")[:, 0].astype(np.uint16).reshape(B0, B1)

    rows = B0 // N_CORES
    in_maps = [
        {"x": u16[c * rows:(c + 1) * rows].reshape(P, VALS_PER_PART).view(np.int16),
         "ident": IDENT}
        for c in range(N_CORES)
    ]
    nc = _get_nc(VALS_PER_PART)
    res = run_bass_kernel_spmd(nc, in_maps, list(range(N_CORES)))

    by = np.empty((B0, B1), dtype=np.uint8)
    for c in range(N_CORES):
        yb = res.results[c]["y"].astype(np.uint8)  # low byte of i16
        by[c * rows:(c + 1) * rows] = yb.reshape(rows, B1)
    out = np.unpackbits(by.reshape(-1, 1), axis=-1).reshape(B0, B1, OBITS)
    return out.astype(in_dtype, copy=False)
